# revision 7
# baseline (speedup 1.0000x reference)
"""LinearAttention Trainium2 kernel (8 NeuronCores, sequence-sharded).

Reference computation (per batch b):
    qkv = x @ W_qkv; q,k,v split; per-head: softmax(q, dim=dh),
    softmax(k, dim=seq); ctx = k^T v; out = q_sm @ ctx; y = out @ W_out + b.

v3 dataflow per core (sequence shard of 1024 rows x 2 batches), ordered
so both AllReduces and all elementwise work hide under the PE stream:
  kv phase (per batch): k,v natural layout (lhsT = xT subtile), exp_k,
      per-tile ctxT/Z single-shot matmuls accumulated in SBUF; AllReduce
      of [ctxT | Z] issued right after each batch. ctx matmuls of tile i
      issue after the kv chain of tile i+1 (software pipelining).
  q phase (per batch): q natural layout; one wide exp ACT per tile,
      per-head sums via segmented DVE tensor_reduce, reciprocal,
      per-head scale split across scalar/vector; q_sm^T produced by
      DMA XBAR transposes (no PE transposes, no PE dependencies at all).
  out phase (per batch): M_h = ctx_h @ W_out_h with 1/Z folded into the
      PSUM->SBUF copy scale (so q_sm^T is used unscaled as lhsT);
      y = sum_t qsmT_t^T @ MZ_t.
Host: shards/transposes/casts x, gathers per-core y shards, adds b_out.
"""
import numpy as np
import ml_dtypes
from contextlib import ExitStack

import concourse.bass as bass
import concourse.mybir as mybir
import concourse.tile as tile
from concourse import bacc
from concourse.bass_utils import run_bass_kernel_spmd

bf16 = ml_dtypes.bfloat16
F32 = mybir.dt.float32
BF = mybir.dt.bfloat16
EXP = mybir.ActivationFunctionType.Exp
COPY = mybir.ActivationFunctionType.Copy
ADD = mybir.AluOpType.add
AX_X = mybir.AxisListType.X

B, N, D = 2, 8192, 1024
H, DH, INNER = 8, 64, 512
NCORES = 8
NL = N // NCORES            # 1024 seq rows per batch per core
SEQ = B * NL                # 2048 rows per core
NT_B = NL // 128            # 8 seq (128-row) tiles per batch


def _body(tc, xT, wq, wo, y):
    nc = tc.nc
    with ExitStack() as ctx:
        const = ctx.enter_context(tc.tile_pool(name="const", bufs=1))
        dram = ctx.enter_context(tc.tile_pool(name="dram", bufs=1, space="DRAM"))

        ones_bf = const.tile([128, 1], BF)
        nc.vector.memset(ones_bf, 1.0)

        # interleave weight-kv and xT loads so kv matmuls start early
        xt = const.tile([128, 8, SEQ], BF)           # resident xT
        wq_sb = const.tile([128, 8, 3 * INNER], BF)
        wo_sb = const.tile([128, 4, D], BF)
        xT_r = xT[:].rearrange("(c p) s -> p c s", p=128)
        for kk in range(8):
            nc.sync.dma_start(out=wq_sb[:, kk, 512:1536],
                              in_=wq[128 * kk:128 * (kk + 1), 512:1536])
            nc.sync.dma_start(out=xt[:, kk, 0:256], in_=xT_r[:, kk, 0:256])
        for kk in range(8):
            nc.sync.dma_start(out=xt[:, kk, 256:NL], in_=xT_r[:, kk, 256:NL])
        for kk in range(8):
            nc.sync.dma_start(out=xt[:, kk, NL:SEQ], in_=xT_r[:, kk, NL:SEQ])
        for kk in range(8):
            nc.sync.dma_start(out=wq_sb[:, kk, 0:512],
                              in_=wq[128 * kk:128 * (kk + 1), 0:512])
        for t in range(4):
            nc.sync.dma_start(out=wo_sb[:, t, :], in_=wo[128 * t:128 * (t + 1), :])

        qsmT = const.tile([128, 4, SEQ], BF)   # persistent q_sm^T
        cz_acc = []
        for b in range(B):
            cz_b = const.tile([128, 260], F32, tag=f"cz{b}", name=f"cz_acc{b}")
            nc.vector.memset(cz_b, 0.0)
            cz_acc.append(cz_b)

        red = []  # allreduced [ctxT | Z] per batch

        # ---- kv phase: k,v + ctx/Z for both batches; ARs issued ASAP ----
        with ExitStack() as pkv:
            kv_ps = pkv.enter_context(tc.tile_pool(name="kv_ps", bufs=2, space="PSUM"))
            cz_ps = pkv.enter_context(tc.tile_pool(name="cz_ps", bufs=2, space="PSUM"))
            ek_pool = pkv.enter_context(tc.tile_pool(name="ek", bufs=3))
            v_pool = pkv.enter_context(tc.tile_pool(name="vp", bufs=3))

            def kv_chain(b, st):
                s0 = b * NL + st * 128
                kv = kv_ps.tile([128, 1024], F32, tag="kv", name="kv")
                for kk in range(8):
                    nc.tensor.matmul(
                        kv[:, 0:512], lhsT=xt[:, kk, s0:s0 + 128],
                        rhs=wq_sb[:, kk, 512:1024],
                        start=(kk == 0), stop=(kk == 7))
                    nc.tensor.matmul(
                        kv[:, 512:1024], lhsT=xt[:, kk, s0:s0 + 128],
                        rhs=wq_sb[:, kk, 1024:1536],
                        start=(kk == 0), stop=(kk == 7))
                expk = ek_pool.tile([128, INNER], BF, tag="expk", name="expk")
                nc.scalar.activation(out=expk, in_=kv[:, 0:512], func=EXP)
                vsb = v_pool.tile([128, INNER], BF, tag="v", name="vsb")
                nc.scalar.copy(out=vsb, in_=kv[:, 512:1024])
                return expk, vsb

            def ctx_mms(b, expk, vsb):
                cz = cz_ps.tile([128, 260], F32, tag="cz", name="cz")
                for h in range(H):
                    t, r = h // 2, h % 2
                    nc.tensor.matmul(
                        cz[64 * r:64 * (r + 1), 64 * t:64 * (t + 1)],
                        lhsT=vsb[:, 64 * h:64 * (h + 1)],
                        rhs=expk[:, 64 * h:64 * (h + 1)],
                        start=True, stop=True)
                for j in range(4):
                    nc.tensor.matmul(
                        cz[:, 256 + j:257 + j],
                        lhsT=expk[:, 128 * j:128 * (j + 1)], rhs=ones_bf,
                        start=True, stop=True)
                nc.vector.tensor_add(cz_acc[b], cz_acc[b], cz)

            prev = None
            for b in range(B):
                for st in range(NT_B):
                    ek_v = kv_chain(b, st)
                    if prev is not None:
                        ctx_mms(prev[0], *prev[1])
                    prev = (b, ek_v)
                # flush the pipeline at batch end so the AR can be issued
                ctx_mms(prev[0], *prev[1])
                prev = None

                part_b = dram.tile([128, 260], F32, tag=f"part{b}", name=f"part{b}")
                red_b = dram.tile([128, 260], F32, tag=f"red{b}", name=f"red{b}")
                nc.sync.dma_start(out=part_b, in_=cz_acc[b])
                nc.gpsimd.collective_compute(
                    "AllReduce", mybir.AluOpType.add,
                    replica_groups=[list(range(NCORES))],
                    ins=[part_b.opt()], outs=[red_b.opt()])
                red.append(red_b)

        # ---- q + out phases share one PSUM scope (2 + 2 + 4 banks) ----
        with ExitStack() as pq:
            q_ps = pq.enter_context(tc.tile_pool(name="q_ps", bufs=2, space="PSUM"))
            m_ps = pq.enter_context(tc.tile_pool(name="m_ps", bufs=2, space="PSUM"))
            y_ps = pq.enter_context(tc.tile_pool(name="y_ps", bufs=4, space="PSUM"))
            eq_pool = pq.enter_context(tc.tile_pool(name="eq", bufs=3))
            qs_pool = pq.enter_context(tc.tile_pool(name="qs", bufs=3))
            qsm_pool = pq.enter_context(tc.tile_pool(name="qsm", bufs=3))
            work2 = pq.enter_context(tc.tile_pool(name="work2", bufs=2))
            small2 = pq.enter_context(tc.tile_pool(name="small2", bufs=2))
            ysb_pool = pq.enter_context(tc.tile_pool(name="ysb", bufs=4))

            # prefetch both allreduce results
            red_sb = []
            for b in range(B):
                red_c = work2.tile([128, 260], F32, tag=f"red{b}", name=f"red_sb{b}")
                nc.sync.dma_start(out=red_c, in_=red[b])
                red_sb.append(red_c)

            def q_tile(b, st):
                s0 = b * NL + st * 128
                qp = q_ps.tile([128, 512], F32, tag="qp", name="qp")
                for kk in range(8):
                    nc.tensor.matmul(
                        qp, lhsT=xt[:, kk, s0:s0 + 128],
                        rhs=wq_sb[:, kk, 0:512],
                        start=(kk == 0), stop=(kk == 7))
                expq = eq_pool.tile([128, 8, 64], BF, tag="eq", name="expq")
                nc.scalar.activation(out=expq, in_=qp, func=EXP)
                qsum = qs_pool.tile([128, 8], F32, tag="qsum", name="qsum")
                nc.vector.tensor_reduce(qsum, expq, axis=AX_X, op=ADD)
                rq = qs_pool.tile([128, 8], F32, tag="rq", name="rq")
                nc.vector.reciprocal(rq, qsum)
                qsm = qsm_pool.tile([128, INNER], BF, tag="qsm", name="qsm")
                for h in range(H):
                    if h % 2 == 0:
                        nc.vector.tensor_scalar_mul(
                            qsm[:, 64 * h:64 * (h + 1)], expq[:, h, :],
                            rq[:, h:h + 1])
                    else:
                        nc.scalar.activation(
                            out=qsm[:, 64 * h:64 * (h + 1)], in_=expq[:, h, :],
                            func=COPY, scale=rq[:, h:h + 1])
                for c in range(4):
                    nc.sync.dma_start(
                        out=qsmT[:, c, s0:s0 + 128],
                        in_=qsm[:, 128 * c:128 * (c + 1)], transpose=True)

            def m_phase(b):
                ctxbf = work2.tile([128, 256], BF, tag="ctxbf", name="ctxbf")
                nc.scalar.copy(out=ctxbf, in_=red_sb[b][:, 0:256])
                rz = small2.tile([128, 4], F32, tag="rz", name="rz")
                nc.vector.reciprocal(rz, red_sb[b][:, 256:260])
                # MZ_h = (ctx_h @ Wout_h) / Z  (1/Z folded into copy scale)
                m_sb = work2.tile([128, 4, D], BF, tag="msb", name="m_sb")
                for t in range(4):
                    for cb in range(2):
                        mp = m_ps.tile([128, 512], F32, tag="mp", name="mp")
                        for r in range(2):
                            nc.tensor.matmul(
                                mp[64 * r:64 * (r + 1), :],
                                lhsT=ctxbf[64 * r:64 * (r + 1), 64 * t:64 * (t + 1)],
                                rhs=wo_sb[64 * r:64 * (r + 1), t, cb * 512:(cb + 1) * 512],
                                start=True, stop=True)
                        if cb == 0:
                            nc.vector.tensor_scalar_mul(
                                m_sb[:, t, 0:512], mp, rz[:, t:t + 1])
                        else:
                            nc.scalar.activation(
                                out=m_sb[:, t, 512:1024], in_=mp,
                                func=COPY, scale=rz[:, t:t + 1])
                return m_sb

            def y_phase(b, m_sb):
                for mi in range(NT_B):
                    ysb = ysb_pool.tile([128, D], F32, tag="ysb", name="ysb")
                    for cb in range(2):
                        yp = y_ps.tile([128, 512], F32, tag="yp", name="yp")
                        for t in range(4):
                            nc.tensor.matmul(
                                yp, lhsT=qsmT[:, t, b * NL + mi * 128:
                                              b * NL + (mi + 1) * 128],
                                rhs=m_sb[:, t, cb * 512:(cb + 1) * 512],
                                start=(t == 0), stop=(t == 3))
                        nc.vector.tensor_copy(
                            out=ysb[:, cb * 512:(cb + 1) * 512], in_=yp)
                    nc.sync.dma_start(
                        out=y[b * NL + mi * 128: b * NL + (mi + 1) * 128, :],
                        in_=ysb)

            for st in range(NT_B):
                q_tile(0, st)
            m0 = m_phase(0)
            for st in range(NT_B):
                q_tile(1, st)
            m1 = m_phase(1)
            y_phase(0, m0)
            y_phase(1, m1)


_COMPILED = None


def _build():
    global _COMPILED
    if _COMPILED is None:
        nc = bacc.Bacc("TRN2", target_bir_lowering=False, debug=False,
                       num_devices=NCORES)
        xT = nc.declare_dram_parameter("xT", [D, SEQ], BF, isOutput=False)
        wq = nc.declare_dram_parameter("wq", [D, 3 * INNER], BF, isOutput=False)
        wo = nc.declare_dram_parameter("wo", [INNER, D], BF, isOutput=False)
        y = nc.declare_dram_parameter("y", [SEQ, D], F32, isOutput=True)
        with tile.TileContext(nc) as tc:
            _body(tc, xT, wq, wo, y)
        nc.compile()
        _COMPILED = nc
    return _COMPILED


def _make_in_maps(x, W_qkv, W_out):
    wq_bf = np.ascontiguousarray(W_qkv).astype(bf16)
    wo_bf = np.ascontiguousarray(W_out).astype(bf16)
    in_maps = []
    for c in range(NCORES):
        rows = slice(c * NL, (c + 1) * NL)
        xs = np.concatenate([x[0, rows], x[1, rows]], axis=0)  # [2048, 1024]
        xT_bf = np.ascontiguousarray(xs.T).astype(bf16)        # [1024, 2048]
        in_maps.append({"xT": xT_bf, "wq": wq_bf, "wo": wo_bf})
    return in_maps


def _run(x, W_qkv, W_out, b_out, trace=False, **spmd_kwargs):
    nc = _build()
    in_maps = _make_in_maps(x, W_qkv, W_out)
    res = run_bass_kernel_spmd(nc, in_maps, list(range(NCORES)),
                               trace=trace, **spmd_kwargs)
    out = np.empty((B, N, D), np.float32)
    for c in range(NCORES):
        yc = res.results[c]["y"]
        rows = slice(c * NL, (c + 1) * NL)
        out[0, rows] = yc[:NL]
        out[1, rows] = yc[NL:]
    out += np.asarray(b_out, np.float32)[None, None, :]
    return out, res


def kernel(x, W_qkv, W_out, b_out):
    x = np.asarray(x, np.float32)
    out, _ = _run(x, np.asarray(W_qkv, np.float32),
                  np.asarray(W_out, np.float32),
                  np.asarray(b_out, np.float32))
    return out
